# revision 1
# baseline (speedup 1.0000x reference)
"""Trainium2 Bass kernel for EnhancedTransformerBlock on ragged graphs.

Layout: transposed activations [channels (partitions), nodes (free)].
Sharding: 64 graphs -> 8 cores x 8 slots, assigned by size-sorted rank so
slot widths (uniform across cores, required for SPMD) hug the max count.
All per-graph segment ops become free-dim slices; GraphNorm stats come from
bn_stats over the zero-padded slot slice plus closed-form corrections.
"""

import math
import numpy as np
import ml_dtypes

N_CORES = 8
B = 64
H = 256
NH = 8
HD = H // NH
EPS = 1e-5
F32 = None  # set on import of mybir below

import concourse.bass as bass
import concourse.bacc as bacc
import concourse.mybir as mybir
import concourse.tile as tile
from concourse.bass_utils import run_bass_kernel_spmd
from contextlib import ExitStack

F32 = mybir.dt.float32
F32R = mybir.dt.float32r
BF16 = mybir.dt.bfloat16
AF = mybir.ActivationFunctionType
OP = mybir.AluOpType

NEG = -1.0e9       # additive key mask (pre-exp)
BIG = 1.0e30       # sumexp offset at padded query columns


def _plan(batch):
    batch = np.asarray(batch).astype(np.int64)
    counts = np.bincount(batch, minlength=B).astype(np.int64)
    starts = np.concatenate([[0], np.cumsum(counts)[:-1]])
    order = np.argsort(-counts, kind="stable")  # rank -> graph id
    NS = B // N_CORES  # slots per core
    Ms, slot_graph = [], np.zeros((N_CORES, NS), np.int64)
    for s in range(NS):
        blk = order[N_CORES * s: N_CORES * s + N_CORES]
        m = int(max(16, math.ceil(max(1, counts[blk].max()) / 16) * 16))
        Ms.append(m)
        for c in range(N_CORES):
            slot_graph[c, s] = blk[c]
    offs = np.concatenate([[0], np.cumsum(Ms)]).astype(np.int64)
    Rtot = int(offs[-1])
    R = int(math.ceil(Rtot / 128) * 128)
    return counts, starts, slot_graph, Ms, offs, Rtot, R


def _qchunks(m):
    # query-range chunks <=256 (PSUM bank budget for the 8-head score tile)
    out = []
    o = 0
    while o < m:
        c = min(256, m - o)
        out.append((o, c))
        o += c
    return out


def _build(nc, Ms, offs, R):
    NS = len(Ms)
    nkt = [math.ceil(m / 128) for m in Ms]
    NKT = sum(nkt)
    SC = 1.0 / math.sqrt(HD)

    # ---- DRAM tensors ----
    d_xt = nc.dram_tensor("xt", [2, 128, R], F32, kind="ExternalInput").ap()
    d_wqk = nc.dram_tensor("wqk", [2, 128, 512], F32R, kind="ExternalInput").ap()
    d_wv = nc.dram_tensor("wv", [2, 128, 256], F32R, kind="ExternalInput").ap()
    d_wo = nc.dram_tensor("wo", [2, 128, 256], F32R, kind="ExternalInput").ap()
    d_w1 = nc.dram_tensor("w1", [2, 128, 1024], F32R, kind="ExternalInput").ap()
    d_w2 = nc.dram_tensor("w2", [8, 128, 256], F32R, kind="ExternalInput").ap()
    # packed per-partition constants, column layout:
    # [qkb(4) ob(2) fb1(8) fb2(2) n1w(2) n1b(2) n2w(2) n2b(2) ga1(NS) gA(NS) gB(NS) km(NKT) ident(128)]
    NCST = 24 + 3 * NS + NKT
    d_cst = nc.dram_tensor("cst", [128, NCST], F32, kind="ExternalInput").ap()
    d_qm = nc.dram_tensor("qm", [1, R], BF16, kind="ExternalInput").ap()        # 0/BIG at padded q
    d_onesb = nc.dram_tensor("onesb", [128, 128], BF16, kind="ExternalInput").ap()
    d_zz = nc.dram_tensor("zz", [128, 2048], F32R, kind="ExternalInput").ap()
    d_ot = nc.dram_tensor("ot", [2, 128, R], F32, kind="ExternalOutput").ap()

    def mm(out, lhsT, rhs, **kw):
        nc.tensor.matmul(out, lhsT, rhs, **kw)

    with tile.TileContext(nc) as tc, ExitStack() as ctx:
        pers = ctx.enter_context(tc.tile_pool(name="pers", bufs=1))
        ptp = ctx.enter_context(tc.tile_pool(name="ptp", bufs=12))
        hgp = ctx.enter_context(tc.tile_pool(name="hgp", bufs=3))
        stat = ctx.enter_context(tc.tile_pool(name="stat", bufs=4))
        psA = ctx.enter_context(tc.tile_pool(name="psA", bufs=4, space="PSUM"))

        # ---- load inputs ----
        def load(name, dram, shape, dt=F32):
            t = pers.tile(shape, dt, tag=name)
            nc.sync.dma_start(out=t, in_=dram)
            return t

        xt = []
        for i in range(2):
            t = pers.tile([128, R], F32, name=f"xt{i}", tag=f"xt{i}")
            nc.sync.dma_start(out=t[:, :R // 2], in_=d_xt[i][:, :R // 2])
            nc.sync.dma_start(out=t[:, R // 2:], in_=d_xt[i][:, R // 2:])
            xt.append(t)
        wqk = [load(f"wqk{i}", d_wqk[i], [128, 512], F32R) for i in range(2)]
        wv = [load(f"wv{i}", d_wv[i], [128, 256], F32R) for i in range(2)]
        wo = [load(f"wo{i}", d_wo[i], [128, 256], F32R) for i in range(2)]
        w1 = [load(f"w1{i}", d_w1[i], [128, 1024], F32R) for i in range(2)]
        w2 = [load(f"w2{i}", d_w2[i], [128, 256], F32R) for i in range(8)]
        cst = load("cst", d_cst, [128, NCST])
        co = 0
        def cslice(n):
            nonlocal co
            a = cst[:, co:co + n]; co += n
            return a
        qkb = [cslice(1) for _ in range(4)]
        ob = [cslice(1) for _ in range(2)]
        fb1 = [cslice(1) for _ in range(8)]
        fb2 = [cslice(1) for _ in range(2)]
        nw = [[cslice(1) for _ in range(2)] for _ in range(2)]
        nb = [[cslice(1) for _ in range(2)] for _ in range(2)]
        ga1 = cslice(NS)
        gA = cslice(NS)
        gB = cslice(NS)
        km = [cslice(1) for _ in range(NKT)]
        qm = load("qm", d_qm, [1, R], BF16)
        onesb = load("onesb", d_onesb, [128, 128], BF16)
        ones1 = onesb[0:1, :]
        oneskt = onesb[:, 0:32]
        qZall = load("qZall", d_zz, [128, 2048], F32R)
        qZ = [qZall[:, 256 * h:256 * h + 256] for h in range(8)]

        NCH = [(o, min(512, R - o)) for o in range(0, R, 512)]

        # ---------- GraphNorm (shared) ----------
        def gnorm(src, dst, widx):
            # per (chtile, slot-half) stats via bn_stats over the padded slice,
            # then corrections for the zero padding (unbiased var, eps on std)
            NH2 = NS // 2
            for ct in range(2):
                for h2 in range(2):
                    sl = range(h2 * NH2, (h2 + 1) * NH2)
                    c0 = h2 * NH2
                    mv = stat.tile([128, 2, NH2], F32, name="mv", tag=f"mv{ct}{h2}")
                    for s in sl:
                        st6 = stat.tile([128, 6], F32, name="st6", tag="st6")
                        nc.vector.bn_stats(out=st6, in_=src[ct][:, offs[s]:offs[s] + Ms[s]])
                        nc.vector.bn_aggr(out=mv[:, :, s - c0:s - c0 + 1], in_=st6)
                    mean_r = mv[:, 0:1, :].squeeze(1)
                    var_r = mv[:, 1:2, :].squeeze(1)
                    m2 = stat.tile([128, NH2], F32, name="m2", tag="m2")
                    nc.vector.tensor_mul(m2, mean_r, mean_r)
                    v1 = stat.tile([128, NH2], F32, name="v1", tag="v1")
                    nc.vector.tensor_mul(v1, var_r, gA[:, c0:c0 + NH2])
                    v2 = stat.tile([128, NH2], F32, name="v2", tag="v2")
                    nc.vector.tensor_mul(v2, m2, gB[:, c0:c0 + NH2])
                    var = stat.tile([128, NH2], F32, name="var", tag="var")
                    nc.vector.tensor_add(var, v1, v2)
                    # std = exp(0.5*ln(var)) + EPS (stays in the exp/ln ACT set)
                    lnv = stat.tile([128, NH2], F32, name="lnv", tag="lnv")
                    nc.scalar.activation(out=lnv, in_=var, func=AF.Ln)
                    std = stat.tile([128, NH2], F32, name="std", tag="std")
                    nc.scalar.activation(out=std, in_=lnv, func=AF.Exp, scale=0.5)
                    nc.vector.tensor_scalar_add(std, std, EPS)
                    rstd = stat.tile([128, NH2], F32, name="rstd", tag="rstd")
                    scr = stat.tile([128, NH2], F32, name="scr", tag="scr")
                    nc.vector.reciprocal_approx_accurate(out=rstd, in_=std, scratch=scr)
                    mean = stat.tile([128, NH2], F32, name="mean", tag="mean")
                    nc.vector.tensor_mul(mean, mean_r, ga1[:, c0:c0 + NH2])
                    scale = stat.tile([128, NH2], F32, name="scale", tag="scale")
                    nc.vector.tensor_scalar_mul(scale, rstd, nw[widx][ct])
                    shift = stat.tile([128, NH2], F32, name="shift", tag="shift")
                    nc.vector.tensor_mul(shift, mean, scale)
                    nc.vector.tensor_scalar(
                        out=shift, in0=shift, scalar1=-1.0, scalar2=nb[widx][ct],
                        op0=OP.mult, op1=OP.add,
                    )
                    for s in sl:
                        nc.vector.tensor_scalar(
                            out=dst[ct][:, offs[s]:offs[s] + Ms[s]],
                            in0=src[ct][:, offs[s]:offs[s] + Ms[s]],
                            scalar1=scale[:, s - c0:s - c0 + 1],
                            scalar2=shift[:, s - c0:s - c0 + 1],
                            op0=OP.mult, op1=OP.add,
                        )

        # ---------- phase 1: gnorm1 ----------
        xn = [pers.tile([128, R], F32R, name=f"xn{i}", tag=f"xn{i}") for i in range(2)]
        Rtot = offs[-1]
        if R > Rtot:
            for ct in range(2):
                nc.sync.dma_start(out=xn[ct][:, Rtot:R], in_=d_zz[:, :R - Rtot])
        gnorm(xt, xn, 0)

        # ---------- phase 2: q,k  (qk[mt] = rows 128*mt of [q;k] = W_qk @ xn) ----
        qk = [pers.tile([128, R], F32R, name=f"qk{m}", tag=f"qk{m}") for m in range(4)]
        for mt in range(4):
            for (o, w) in NCH:
                ps = psA.tile([128, 512], F32, name="ps1", tag="ps1")
                for kt in range(2):
                    mm(ps[:, :w], wqk[kt][:, 128 * mt:128 * mt + 128],
                       xn[kt][:, o:o + w], start=(kt == 0), stop=(kt == 1))
                nc.scalar.activation(out=qk[mt][:, o:o + w], in_=ps[:, :w],
                                     func=AF.Identity, bias=qkb[mt])
        # ---------- phase 2b: vRows per (slot, ktile)  [keys, 256] ----------
        vr = pers.tile([128, 256 * NKT], BF16, name="vr", tag="vr")
        vri = {}
        idx = 0
        for s in range(NS):
            for kt in range(nkt[s]):
                vri[(s, kt)] = idx
                mkt = min(128, Ms[s] - 128 * kt)
                ko = offs[s] + 128 * kt
                ps = psA.tile([128, 512], F32, name="ps1", tag="ps1")
                for ct in range(2):
                    mm(ps[:mkt, :256], xn[ct][:, ko:ko + mkt], wv[ct],
                       start=(ct == 0), stop=(ct == 1))
                nc.vector.tensor_copy(vr[:mkt, 256 * idx:256 * idx + 256], ps[:mkt, :256])
                idx += 1

        # ---------- phase 3: attention per (slot, qchunk) ----------
        ctxt = [pers.tile([128, R], F32R, name=f"ctx{i}", tag=f"ctx{i}") for i in range(2)]
        if R > Rtot:
            for ct in range(2):
                nc.sync.dma_start(out=ctxt[ct][:, Rtot:R], in_=d_zz[:, :R - Rtot])
        kmi = {}
        idx = 0
        for s in range(NS):
            for kt in range(nkt[s]):
                kmi[(s, kt)] = idx
                idx += 1
        with tc.tile_pool(name="psST", bufs=2, space="PSUM") as psST:
            for s in range(NS):
                for (qo, qc) in _qchunks(Ms[s]):
                    qbase = offs[s] + qo
                    for h in range(8):
                        hp = 32 * (h % 4)
                        nc.vector.tensor_copy(
                            qZ[h][hp:hp + 32, :qc],
                            qk[h // 4][hp:hp + 32, qbase:qbase + qc])
                    pts = []
                    for kt in range(nkt[s]):
                        mkt = min(128, Ms[s] - 128 * kt)
                        ko = offs[s] + 128 * kt
                        ph = []
                        for g in range(2):
                            st = psST.tile([128, 4 * 256], F32, name="st", tag="st")
                            for j in range(4):
                                h = 4 * g + j
                                lhsT = qk[2 + h // 4][:, ko:ko + mkt]
                                mm(st[:mkt, j * qc:(j + 1) * qc], lhsT, qZ[h][:, :qc],
                                   start=True, stop=True)
                            pt = ptp.tile([128, 4 * 256], BF16, name="pt", tag="pt")
                            nc.scalar.activation(
                                out=pt[:mkt, :4 * qc], in_=st[:mkt, :4 * qc],
                                func=AF.Exp, bias=km[kmi[(s, kt)]][:mkt], scale=SC)
                            ph.append(pt)
                        pts.append(ph)
                    cs = [psA.tile([128, 512], F32, name="cs", tag="ps1") for _ in range(2)]
                    for g in range(2):
                        mm(cs[g][:, qc:2 * qc], ones1[:, :128], qm[:, qbase:qbase + qc],
                           start=True, stop=False)
                    for kt in range(nkt[s]):
                        mkt = min(128, Ms[s] - 128 * kt)
                        vb = 256 * vri[(s, kt)]
                        last = kt == nkt[s] - 1
                        for g in range(2):
                            for j in range(4):
                                h = 4 * g + j
                                mm(cs[g][32 * j:32 * j + 32, 0:qc],
                                   vr[:mkt, vb + 32 * h:vb + 32 * h + 32],
                                   pts[kt][g][:mkt, j * qc:(j + 1) * qc],
                                   start=(kt == 0), stop=last, tile_position=(0, 32 * j))
                                mm(cs[g][32 * j:32 * j + 32, qc:2 * qc],
                                   oneskt[:mkt, :],
                                   pts[kt][g][:mkt, j * qc:(j + 1) * qc],
                                   start=False, stop=last, tile_position=(0, 32 * j))
                    for g in range(2):
                        rec = stat.tile([128, 256], F32, name="rec", tag="rec")
                        nc.vector.reciprocal_approx_fast(out=rec[:, :qc], in_=cs[g][:, qc:2 * qc])
                        nc.vector.tensor_mul(
                            ctxt[g][:, qbase:qbase + qc], cs[g][:, 0:qc], rec[:, :qc])

        # ---------- phase 4: out_proj + residual -> x2 ----------
        x2 = [pers.tile([128, R], F32, name=f"x2{i}", tag=f"x2{i}") for i in range(2)]
        for ct in range(2):
            for (o, w) in NCH:
                ps = psA.tile([128, 512], F32, name="ps1", tag="ps1")
                for kt in range(2):
                    mm(ps[:, :w], wo[kt][:, 128 * ct:128 * ct + 128],
                       ctxt[kt][:, o:o + w], start=(kt == 0), stop=(kt == 1))
                nc.vector.scalar_tensor_tensor(
                    out=x2[ct][:, o:o + w], in0=ps[:, :w], scalar=ob[ct],
                    in1=xt[ct][:, o:o + w], op0=OP.add, op1=OP.add,
                )

        # ---------- phase 5: gnorm2 ----------
        # reuse xn slots; dead zone is still zero from phase 1
        xn2 = [pers.tile([128, R], F32R, name=f"xn{i}", tag=f"xn{i}") for i in range(2)]
        gnorm(x2, xn2, 1)

        # ---------- phase 6: FFN ----------
        out_t = [pers.tile([128, R], F32, name=f"xt{i}", tag=f"xt{i}") for i in range(2)]  # reuse xt slots
        half = int(offs[NS // 2])
        HCH = []
        for lo, hi in ((0, half), (half, R)):
            o = lo
            while o < hi:
                w = min(1024, hi - o)
                HCH.append((o, w))
                o += w
        with tc.tile_pool(name="psH", bufs=2, space="PSUM") as psH:
            for (o, w) in HCH:
                hg = []
                for mt in range(8):
                    ps = psH.tile([128, 1024], F32, name="hps", tag="hps")
                    for o2 in range(0, w, 512):
                        wc = min(512, w - o2)
                        for kt in range(2):
                            mm(ps[:, o2:o2 + wc], w1[kt][:, 128 * mt:128 * mt + 128],
                               xn2[kt][:, o + o2:o + o2 + wc],
                               start=(kt == 0), stop=(kt == 1))
                    h = hgp.tile([128, 1024], F32R, name="hg", tag="hg")
                    nc.scalar.activation(out=h[:, :w], in_=ps[:, :w],
                                         func=AF.Gelu, bias=fb1[mt])
                    hg.append(h)
                for ct in range(2):
                    for o2 in range(0, w, 512):
                        w2c = min(512, w - o2)
                        ps2 = psA.tile([128, 512], F32, name="ps2", tag="ps1")
                        for kt in range(8):
                            mm(ps2[:, :w2c], w2[kt][:, 128 * ct:128 * ct + 128],
                               hg[kt][:, o2:o2 + w2c], start=(kt == 0), stop=(kt == 7))
                        nc.vector.scalar_tensor_tensor(
                            out=out_t[ct][:, o + o2:o + o2 + w2c], in0=ps2[:, :w2c],
                            scalar=fb2[ct], in1=x2[ct][:, o + o2:o + o2 + w2c],
                            op0=OP.add, op1=OP.add)
                        nc.sync.dma_start(out=d_ot[ct][:, o + o2:o + o2 + w2c],
                                          in_=out_t[ct][:, o + o2:o + o2 + w2c])
    return nc


_CACHE = {}


def _prepare(inputs):
    x = np.asarray(inputs["x"], np.float32)
    batch = np.asarray(inputs["batch"]).astype(np.int64)
    counts, starts, slot_graph, Ms, offs, Rtot, R = _plan(batch)
    NS = len(Ms)
    nkt = [math.ceil(m / 128) for m in Ms]
    NKT = sum(nkt)

    in_proj_w = np.asarray(inputs["in_proj_w"], np.float32)
    in_proj_b = np.asarray(inputs["in_proj_b"], np.float32)
    out_proj_w = np.asarray(inputs["out_proj_w"], np.float32)
    out_proj_b = np.asarray(inputs["out_proj_b"], np.float32)
    ffn_w1 = np.asarray(inputs["ffn_w1"], np.float32)
    ffn_b1 = np.asarray(inputs["ffn_b1"], np.float32)
    ffn_w2 = np.asarray(inputs["ffn_w2"], np.float32)
    ffn_b2 = np.asarray(inputs["ffn_b2"], np.float32)

    # fold the v-branch input bias through out_proj (exact, linear)
    ob_eff = out_proj_b + out_proj_w @ in_proj_b[2 * H:3 * H]

    wqk = np.ascontiguousarray(in_proj_w[:2 * H].T.reshape(2, 128, 512))
    wv = np.ascontiguousarray(in_proj_w[2 * H:].T.reshape(2, 128, 256))
    wo = np.ascontiguousarray(out_proj_w.T.reshape(2, 128, 256))
    w1 = np.ascontiguousarray(ffn_w1.T.reshape(2, 128, 1024))
    w2 = np.ascontiguousarray(ffn_w2.T.reshape(8, 128, 256))
    qkb = np.ascontiguousarray(in_proj_b[:2 * H].reshape(4, 128, 1))
    ob = np.ascontiguousarray(ob_eff.reshape(2, 128, 1))
    fb1 = np.ascontiguousarray(ffn_b1.reshape(8, 128, 1))
    fb2 = np.ascontiguousarray(ffn_b2.reshape(2, 128, 1))
    nw = np.stack([np.asarray(inputs["norm1_w"], np.float32).reshape(2, 128, 1),
                   np.asarray(inputs["norm2_w"], np.float32).reshape(2, 128, 1)])
    nb = np.stack([np.asarray(inputs["norm1_b"], np.float32).reshape(2, 128, 1),
                   np.asarray(inputs["norm2_b"], np.float32).reshape(2, 128, 1)])

    xT = x.T  # [256, N]
    xts = np.zeros((N_CORES, 2, 128, R), np.float32)
    ga1 = np.zeros((N_CORES, 128, NS), np.float32)
    gA = np.zeros((N_CORES, 128, NS), np.float32)
    gB = np.zeros((N_CORES, 128, NS), np.float32)
    kms = np.full((N_CORES, NKT, 128, 1), NEG, np.float32)
    qms = np.zeros((N_CORES, 1, R), np.float32)
    onesb = np.ones((128, 128), ml_dtypes.bfloat16)
    zz = np.zeros((128, 2048), np.float32)
    for c in range(N_CORES):
        for s in range(NS):
            g = slot_graph[c, s]
            n = int(counts[g])
            st = int(starts[g])
            o = int(offs[s])
            if n > 0:
                blk = xT[:, st:st + n]
                xts[c, 0, :, o:o + n] = blk[:128]
                xts[c, 1, :, o:o + n] = blk[128:]
            ne = max(n, 1)
            ga1[c, :, s] = Ms[s] / ne
            inv_nm1 = 1.0 / max(ne - 1, 1)
            gA[c, :, s] = Ms[s] * inv_nm1
            gB[c, :, s] = Ms[s] * (1.0 - Ms[s] / ne) * inv_nm1
            ki = sum(nkt[:s])
            for kt in range(nkt[s]):
                v = min(128, max(0, n - 128 * kt))
                kms[c, ki + kt, :v, 0] = 0.0
            qms[c, 0, o + n:o + Ms[s]] = BIG
        qms[c, 0, Rtot:R] = BIG

    key = (tuple(Ms), R)
    if key not in _CACHE:
        nc = bacc.Bacc("TRN2", target_bir_lowering=False, debug=False,
                       num_devices=N_CORES)
        _build(nc, Ms, offs, R)
        nc.compile()
        _CACHE[key] = nc
    nc = _CACHE[key]

    in_maps = []
    for c in range(N_CORES):
        in_maps.append({
            "xt": xts[c], "wqk": wqk, "wv": wv, "wo": wo, "w1": w1, "w2": w2,
            "cst": np.ascontiguousarray(np.concatenate(
                [qkb[:, :, 0].T, ob[:, :, 0].T, fb1[:, :, 0].T, fb2[:, :, 0].T,
                 nw.reshape(4, 128).T, nb.reshape(4, 128).T,
                 ga1[c], gA[c], gB[c],
                 kms[c][:, :, 0].T], axis=1).astype(np.float32)),
            "qm": qms[c].astype(ml_dtypes.bfloat16),
            "onesb": onesb, "zz": zz,
        })

    def unpack(outs):
        out = np.empty((x.shape[0], H), np.float32)
        for c in range(N_CORES):
            ot = outs[c]["ot"]  # [2, 128, R]
            full = np.concatenate([ot[0], ot[1]], axis=0)  # [256, R]
            for s in range(NS):
                g = slot_graph[c, s]
                n = int(counts[g])
                st = int(starts[g])
                o = int(offs[s])
                if n > 0:
                    out[st:st + n] = full[:, o:o + n].T
        return out

    return nc, in_maps, unpack


def kernel(**inputs):
    nc, in_maps, unpack = _prepare(inputs)
    res = run_bass_kernel_spmd(nc, in_maps, list(range(N_CORES)))
    return unpack(res.results)


def _traced_run(**inputs):
    """Cost-model timeline (single core) + warm wall-clock. Returns model ns."""
    import time
    nc, in_maps, unpack = _prepare(inputs)
    t0 = time.time()
    run_bass_kernel_spmd(nc, in_maps, list(range(N_CORES)))
    t1 = time.time()
    run_bass_kernel_spmd(nc, in_maps, list(range(N_CORES)))
    t2 = time.time()
    print(f"wall cold: {t1 - t0:.2f}s  warm: {t2 - t1:.2f}s")
    from concourse.timeline_sim import TimelineSim
    import trails.perfetto as _tp
    for _m in ("enable_explicit_ordering", "reserve_process_order",
               "reserve_thread_order", "set_process_order", "set_thread_order",
               "add_instant"):
        if not hasattr(_tp.LazyPerfetto, _m):
            setattr(_tp.LazyPerfetto, _m, lambda self, *a, **k: None)
    if not hasattr(_tp.LazyPerfetto, "add_counter"):
        def _add_counter(self, *a, **k):
            try:
                self.update_counter(*a, **k)
            except Exception:
                pass
        _tp.LazyPerfetto.add_counter = _add_counter
    tl = TimelineSim(nc, trace=True)
    total = tl.simulate()
    pf = tl.perfetto
    if callable(pf):
        pf = pf()
    if pf is not None:
        try:
            pf.save("/root/problem/tl.perfetto-trace")
        except Exception as e:
            print("perfetto dump failed:", e)
    return total

